# revision 14
# baseline (speedup 1.0000x reference)
"""Trainium2 Bass kernel for nn_AttentionLayer: full attention layer
(QKV proj -> sigmoid-gated scores -> softmax -> AV -> out proj), SPMD over
8 NeuronCores: batch b = core//4, head-group hg = core%4 (4 heads each).

Returns (output, attn_weights) matching the reference.

Self-contained: hardcodes all shapes; does not read any sibling files.
"""
import sys

sys.path.insert(0, "/opt/trn_rl_repo")

import numpy as np

# ---------------------------------------------------------------- constants
B = 2
QL = 2048
KL = 2048
D = 1024
NH = 16
DH = 64
NCORES = 8
HPC = NH // 4          # 4 heads per core
CPC = HPC * DH         # 256 cols per core

# Matmul operand dtype per stage: "f32" (exact, 4 cyc/row) or "f32r"
# (TF32-like, ~1.3e-4 rel err, 1 cyc/row).
MM_CFG = {
    "proj": "f32r",
    "qk": "f32r",
    "av": "f32r",
    "out": "f32r",
}

_CACHE = {}


def _build(cfg_key, reps=1):
    import concourse.bacc as bacc
    import concourse.tile as tile
    from concourse import mybir
    from concourse.masks import make_identity
    from contextlib import ExitStack

    f32 = mybir.dt.float32
    f32r = mybir.dt.float32r
    AF = mybir.ActivationFunctionType

    def mmdt(stage):
        return f32r if MM_CFG[stage] == "f32r" else f32

    nc = bacc.Bacc(None)

    # ------------------------------------------------------------- dram I/O
    QT = nc.declare_dram_parameter("QT", [D, QL], f32, isOutput=False)
    KT = nc.declare_dram_parameter("KT", [D, KL], f32, isOutput=False)
    Wb = nc.declare_dram_parameter("Wb", [QL, KL], f32, isOutput=False)
    wq_d = nc.declare_dram_parameter("wq", [D, CPC], f32, isOutput=False)
    wk_d = nc.declare_dram_parameter("wk", [D, CPC], f32, isOutput=False)
    wv_d = nc.declare_dram_parameter("wv", [D, CPC], f32, isOutput=False)
    wo_d = nc.declare_dram_parameter("wo", [CPC, D], f32, isOutput=False)
    bq_d = nc.declare_dram_parameter("bq2", [128, 2], f32, isOutput=False)
    bk_d = nc.declare_dram_parameter("bk2", [128, 2], f32, isOutput=False)
    attnw = nc.declare_dram_parameter("attnw", [HPC * QL, KL], f32, isOutput=True)
    partial = nc.declare_dram_parameter("partial", [QL, D], f32, isOutput=True)

    with tile.TileContext(nc) as tc:
      for _rep in range(reps):
       with ExitStack() as ctx:
        singles = ctx.enter_context(tc.tile_pool(name="singles", bufs=1))

        qk_dt = mmdt("qk")
        av_dt = mmdt("av")
        out_dt = mmdt("out")
        pr_dt = mmdt("proj")

        # persistent tiles
        qpt = [singles.tile([128, QL], qk_dt, tag=f"qpt{t}", name=f"qpt{t}") for t in range(2)]
        kpt = [singles.tile([128, KL], qk_dt, tag=f"kpt{t}", name=f"kpt{t}") for t in range(2)]
        vp = [singles.tile([128, CPC], av_dt, tag=f"vp{t}", name=f"vp{t}") for t in range(16)]
        aot = [singles.tile([128, QL], out_dt, tag=f"aot{t}", name=f"aot{t}") for t in range(2)]
        woc = singles.tile([128, 2, D], out_dt, tag="woc")
        ident = singles.tile([128, 128], f32, tag="ident")
        bq_sb = singles.tile([128, 2], f32, tag="bq")
        bk_sb = singles.tile([128, 2], f32, tag="bk")

        make_identity(nc, ident)
        nc.sync.dma_start(out=bq_sb, in_=bq_d[:, :])
        nc.sync.dma_start(out=bk_sb, in_=bk_d[:, :])

        # ------------------------------------------------------ phase 1: proj
        with tc.tile_pool(name="p1sb", bufs=1) as p1sb, \
             tc.tile_pool(name="p1ps", bufs=2, space="PSUM") as p1ps:
            # weights: load raw in a transient sub-pool, round into p1sb
            wq_sb = p1sb.tile([128, 8, CPC], pr_dt, tag="wq_r")
            wk_sb = p1sb.tile([128, 8, CPC], pr_dt, tag="wk_r")
            wv_sb = p1sb.tile([128, 8, CPC], pr_dt, tag="wv_r")
            with tc.tile_pool(name="p1w", bufs=1) as p1w:
                wq_raw = p1w.tile([128, 8, CPC], f32, tag="wq_raw")
                wk_raw = p1w.tile([128, 8, CPC], f32, tag="wk_raw")
                wv_raw = p1w.tile([128, 8, CPC], f32, tag="wv_raw")
                wo_raw = p1w.tile([128, 2, D], f32, tag="wo_raw")
                nc.sync.dma_start(out=wq_raw, in_=wq_d.rearrange("(i p) c -> p i c", p=128))
                nc.sync.dma_start(out=wk_raw, in_=wk_d.rearrange("(i p) c -> p i c", p=128))
                nc.sync.dma_start(out=wv_raw, in_=wv_d.rearrange("(i p) c -> p i c", p=128))
                nc.sync.dma_start(out=wo_raw, in_=wo_d.rearrange("(i p) c -> p i c", p=128))
                nc.vector.tensor_copy(wq_sb, wq_raw)
                nc.vector.tensor_copy(wk_sb, wk_raw)
                nc.vector.tensor_copy(wv_sb, wv_raw)
                nc.vector.tensor_copy(woc, wo_raw)

            SC = 256  # s-chunk width
            # K side first: KpT + Vp (the first QK needs all of KpT)
            for sc in range(KL // SC):
                kraw = p1sb.tile([128, 8, SC], f32, tag="kraw", bufs=2)
                nc.sync.dma_start(
                    out=kraw,
                    in_=KT.rearrange("(i p) s -> p i s", p=128)[:, :, sc * SC:(sc + 1) * SC])
                if pr_dt == f32:
                    kch = kraw
                else:
                    kch = p1sb.tile([128, 8, SC], pr_dt, tag="kch", bufs=2)
                    nc.vector.tensor_copy(kch, kraw)
                for t in range(2):
                    ps2 = p1ps.tile([128, SC], f32, tag="pp")
                    for ki in range(8):
                        nc.tensor.matmul(
                            ps2,
                            lhsT=wk_sb[:, ki, t * 128:(t + 1) * 128],
                            rhs=kch[:, ki, :],
                            start=(ki == 0), stop=(ki == 7))
                    nc.scalar.activation(
                        out=kpt[t][:, sc * SC:(sc + 1) * SC], in_=ps2,
                        func=AF.Identity, bias=bk_sb[:, t:t + 1], scale=1.0)
                for st in range(SC // 128):
                    psv = p1ps.tile([128, CPC], f32, tag="ppv")
                    for ki in range(8):
                        nc.tensor.matmul(
                            psv,
                            lhsT=kch[:, ki, st * 128:(st + 1) * 128],
                            rhs=wv_sb[:, ki, :],
                            start=(ki == 0), stop=(ki == 7))
                    nc.vector.tensor_copy(vp[sc * (SC // 128) + st], psv)
            # Q side
            for sc in range(QL // SC):
                qraw = p1sb.tile([128, 8, SC], f32, tag="qraw", bufs=2)
                nc.sync.dma_start(
                    out=qraw,
                    in_=QT.rearrange("(i p) s -> p i s", p=128)[:, :, sc * SC:(sc + 1) * SC])
                if pr_dt == f32:
                    qch = qraw
                else:
                    qch = p1sb.tile([128, 8, SC], pr_dt, tag="qch", bufs=2)
                    nc.vector.tensor_copy(qch, qraw)
                for t in range(2):
                    ps = p1ps.tile([128, SC], f32, tag="pp")
                    for ki in range(8):
                        nc.tensor.matmul(
                            ps,
                            lhsT=wq_sb[:, ki, t * 128:(t + 1) * 128],
                            rhs=qch[:, ki, :],
                            start=(ki == 0), stop=(ki == 7))
                    nc.scalar.activation(
                        out=qpt[t][:, sc * SC:(sc + 1) * SC], in_=ps,
                        func=AF.Identity, bias=bq_sb[:, t:t + 1], scale=1.0)

        # ------------------------------------------------- phase 2: attention
        with tc.tile_pool(name="m2sb", bufs=1) as m2, \
             tc.tile_pool(name="m2ps", bufs=1, space="PSUM") as pp:
            for qc in range(4):
                mtiles = []
                for qt in range(4):
                    w_t = m2.tile([128, KL], f32, tag="wm", bufs=4)
                    qi = qc * 4 + qt
                    nc.sync.dma_start(out=w_t, in_=Wb[qi * 128:(qi + 1) * 128, :])
                    nc.scalar.activation(out=w_t, in_=w_t, func=AF.Sigmoid)
                    mtiles.append(w_t)
                for h in range(4):
                    hb = (h % 2) * 64
                    ht = h // 2
                    den_t = m2.tile([128, 4], f32, tag="den", bufs=2)
                    rec_t = m2.tile([128, 4], f32, tag="rec", bufs=2)
                    a_tiles = []
                    for qt in range(4):
                        qi = qc * 4 + qt
                        sm_t = m2.tile([128, KL], f32, tag="sm", bufs=2)
                        for half in range(2):
                            sp = pp.tile([128, 1024], f32, tag="sp", bufs=2)
                            for kc in range(2):
                                nc.tensor.matmul(
                                    sp[:, kc * 512:(kc + 1) * 512],
                                    lhsT=qpt[ht][hb:hb + 64, qi * 128:(qi + 1) * 128],
                                    rhs=kpt[ht][hb:hb + 64,
                                                half * 1024 + kc * 512:half * 1024 + (kc + 1) * 512],
                                    start=True, stop=True)
                            nc.vector.tensor_mul(
                                sm_t[:, half * 1024:(half + 1) * 1024],
                                sp, mtiles[qt][:, half * 1024:(half + 1) * 1024])
                        nc.scalar.activation(
                            out=sm_t, in_=sm_t, func=AF.Exp, scale=0.125,
                            accum_out=den_t[:, qt:qt + 1])
                        nc.vector.reciprocal(rec_t[:, qt:qt + 1], den_t[:, qt:qt + 1])
                        a_t = m2.tile([128, KL], f32, tag="a", bufs=8)
                        nc.vector.tensor_scalar_mul(a_t, sm_t, rec_t[:, qt:qt + 1])
                        nc.sync.dma_start(
                            out=attnw[h * QL + qi * 128:h * QL + (qi + 1) * 128, :],
                            in_=a_t)
                        a_tiles.append(a_t)
                    av = pp.tile([64, 512], f32, tag="av", bufs=1)
                    ci = 0
                    for kt in range(16):
                        at_t = m2.tile([128, 4, 128], av_dt, tag="at", bufs=3)
                        for pair in range(2):
                            atp = pp.tile([128, 1024], f32, tag="atp", bufs=1)
                            for j in range(2):
                                qt = pair * 2 + j
                                nc.tensor.transpose(
                                    out=atp[:, j * 512:j * 512 + 128],
                                    in_=a_tiles[qt][:, kt * 128:(kt + 1) * 128],
                                    identity=ident)
                            src = atp.rearrange("p (two c) -> p two c", two=2)[:, :, 0:128]
                            dst = at_t[:, pair * 2:pair * 2 + 2, :]
                            ci += 1
                            if ci % 9 < 4:
                                nc.vector.tensor_copy(dst, src)
                            else:
                                nc.scalar.copy(out=dst, in_=src)
                        nc.tensor.matmul(
                            av, lhsT=vp[kt][:, h * 64:(h + 1) * 64],
                            rhs=at_t.rearrange("p four c -> p (four c)"),
                            start=(kt == 0), stop=(kt == 15))
                    nc.scalar.copy(
                        out=aot[ht][hb:hb + 64, qc * 512:(qc + 1) * 512], in_=av)
                # out-projection for this qc's query tiles (aot cols final now)
                for st4 in range(4):
                    st = qc * 4 + st4
                    o_t = m2.tile([128, D], f32, tag="o", bufs=2)
                    for nb in range(2):
                        op = pp.tile([128, 512], f32, tag="op", bufs=1)
                        for ct in range(2):
                            nc.tensor.matmul(
                                op, lhsT=aot[ct][:, st * 128:(st + 1) * 128],
                                rhs=woc[:, ct, nb * 512:(nb + 1) * 512],
                                start=(ct == 0), stop=(ct == 1))
                        if nb == 0:
                            nc.vector.tensor_copy(o_t[:, 0:512], op)
                        else:
                            nc.scalar.copy(out=o_t[:, 512:1024], in_=op)
                    nc.sync.dma_start(out=partial[st * 128:(st + 1) * 128, :], in_=o_t)


    nc.finalize()
    return nc


def _get_nc(reps=1):
    key = (tuple(sorted(MM_CFG.items())), reps)
    if key not in _CACHE:
        _CACHE[key] = _build(key, reps=reps)
    return _CACHE[key]


def kernel(Q, K, W, Wq, bq, Wk, bk, Wv, bv, Wo, bo):
    from concourse.bass_utils import run_bass_kernel_spmd

    Q = np.asarray(Q, dtype=np.float32)
    K = np.asarray(K, dtype=np.float32)
    W = np.asarray(W, dtype=np.float32)
    Wq = np.asarray(Wq, dtype=np.float32)
    bq = np.asarray(bq, dtype=np.float32)
    Wk = np.asarray(Wk, dtype=np.float32)
    bk = np.asarray(bk, dtype=np.float32)
    Wv = np.asarray(Wv, dtype=np.float32)
    bv = np.asarray(bv, dtype=np.float32)
    Wo = np.asarray(Wo, dtype=np.float32)
    bo = np.asarray(bo, dtype=np.float32)

    nc = _get_nc()

    in_maps = []
    for c in range(NCORES):
        b = c // 4
        hg = c % 4
        cs = slice(hg * CPC, (hg + 1) * CPC)
        in_maps.append({
            "QT": np.ascontiguousarray(Q[b].T),
            "KT": np.ascontiguousarray(K[b].T),
            "Wb": np.ascontiguousarray(W[b]),
            "wq": np.ascontiguousarray(Wq[:, cs]),
            "wk": np.ascontiguousarray(Wk[:, cs]),
            "wv": np.ascontiguousarray(Wv[:, cs]),
            "wo": np.ascontiguousarray(Wo[cs, :]),
            "bq2": np.ascontiguousarray(bq[cs].reshape(2, 128).T),
            "bk2": np.ascontiguousarray(bk[cs].reshape(2, 128).T),
        })

    global LAST_IN_MAPS
    LAST_IN_MAPS = in_maps
    res = run_bass_kernel_spmd(nc, in_maps, list(range(NCORES))).results

    bo_full = bo + bv @ Wo
    output = np.empty((B, QL, D), np.float32)
    attn = np.empty((B, NH, QL, KL), np.float32)
    for b in range(B):
        acc = None
        for hg in range(4):
            c = b * 4 + hg
            p = res[c]["partial"]
            acc = p.copy() if acc is None else acc + p
            attn[b, hg * HPC:(hg + 1) * HPC] = res[c]["attnw"].reshape(HPC, QL, KL)
        output[b] = acc + bo_full[None, :]
    return output, attn


# revision 15
# speedup vs baseline: 1.8348x; 1.8348x over previous
"""Trainium2 Bass kernel for nn_AttentionLayer: full attention layer
(QKV proj -> sigmoid-gated scores -> softmax -> AV -> out proj), SPMD over
8 NeuronCores: batch b = core//4, head-group hg = core%4 (4 heads each).

Returns (output, attn_weights) matching the reference.

Self-contained: hardcodes all shapes; does not read any sibling files.
"""
import sys

sys.path.insert(0, "/opt/trn_rl_repo")

import numpy as np

# ---------------------------------------------------------------- constants
B = 2
QL = 2048
KL = 2048
D = 1024
NH = 16
DH = 64
NCORES = 8
HPC = NH // 4          # 4 heads per core
CPC = HPC * DH         # 256 cols per core

# Matmul operand dtype per stage: "f32" (exact, 4 cyc/row) or "f32r"
# (TF32-like, ~1.3e-4 rel err, 1 cyc/row).
MM_CFG = {
    "proj": "f32r",
    "qk": "f32r",
    "av": "f32r",
    "out": "f32r",
}

_CACHE = {}


def _build(cfg_key, reps=1):
    import concourse.bacc as bacc
    import concourse.tile as tile
    from concourse import mybir
    from concourse.masks import make_identity
    from contextlib import ExitStack

    f32 = mybir.dt.float32
    f32r = mybir.dt.float32r
    AF = mybir.ActivationFunctionType

    def mmdt(stage):
        return f32r if MM_CFG[stage] == "f32r" else f32

    nc = bacc.Bacc(None)

    # ------------------------------------------------------------- dram I/O
    QT = nc.declare_dram_parameter("QT", [D, QL], f32, isOutput=False)
    KT = nc.declare_dram_parameter("KT", [D, KL], f32, isOutput=False)
    Wb = nc.declare_dram_parameter("Wb", [QL, KL], f32, isOutput=False)
    wq_d = nc.declare_dram_parameter("wq", [D, CPC], f32, isOutput=False)
    wk_d = nc.declare_dram_parameter("wk", [D, CPC], f32, isOutput=False)
    wv_d = nc.declare_dram_parameter("wv", [D, CPC], f32, isOutput=False)
    wo_d = nc.declare_dram_parameter("wo", [CPC, D], f32, isOutput=False)
    bq_d = nc.declare_dram_parameter("bq2", [128, 2], f32, isOutput=False)
    bk_d = nc.declare_dram_parameter("bk2", [128, 2], f32, isOutput=False)
    attnw = nc.declare_dram_parameter("attnw", [HPC * QL, KL], f32, isOutput=True)
    partial = nc.declare_dram_parameter("partial", [QL, D], f32, isOutput=True)

    with tile.TileContext(nc) as tc:
      for _rep in range(reps):
       with ExitStack() as ctx:
        singles = ctx.enter_context(tc.tile_pool(name="singles", bufs=1))

        qk_dt = mmdt("qk")
        av_dt = mmdt("av")
        out_dt = mmdt("out")
        pr_dt = mmdt("proj")

        # persistent tiles
        qpt = [singles.tile([128, QL], qk_dt, tag=f"qpt{t}", name=f"qpt{t}") for t in range(2)]
        kpt = [singles.tile([128, KL], qk_dt, tag=f"kpt{t}", name=f"kpt{t}") for t in range(2)]
        vp = [singles.tile([128, CPC], av_dt, tag=f"vp{t}", name=f"vp{t}") for t in range(16)]
        aot = [singles.tile([128, QL], out_dt, tag=f"aot{t}", name=f"aot{t}") for t in range(2)]
        woc = singles.tile([128, 2, D], out_dt, tag="woc")
        ident = singles.tile([128, 128], f32, tag="ident")
        bq_sb = singles.tile([128, 2], f32, tag="bq")
        bk_sb = singles.tile([128, 2], f32, tag="bk")

        make_identity(nc, ident)
        nc.sync.dma_start(out=bq_sb, in_=bq_d[:, :])
        nc.sync.dma_start(out=bk_sb, in_=bk_d[:, :])

        # ------------------------------------------------------ phase 1: proj
        with tc.tile_pool(name="p1sb", bufs=1) as p1sb, \
             tc.tile_pool(name="p1ps", bufs=2, space="PSUM") as p1ps:
            # weights: load raw in a transient sub-pool, round into p1sb
            wq_sb = p1sb.tile([128, 8, CPC], pr_dt, tag="wq_r")
            wk_sb = p1sb.tile([128, 8, CPC], pr_dt, tag="wk_r")
            wv_sb = p1sb.tile([128, 8, CPC], pr_dt, tag="wv_r")
            with tc.tile_pool(name="p1w", bufs=1) as p1w:
                wq_raw = p1w.tile([128, 8, CPC], f32, tag="wq_raw")
                wk_raw = p1w.tile([128, 8, CPC], f32, tag="wk_raw")
                wv_raw = p1w.tile([128, 8, CPC], f32, tag="wv_raw")
                wo_raw = p1w.tile([128, 2, D], f32, tag="wo_raw")
                nc.sync.dma_start(out=wq_raw, in_=wq_d.rearrange("(i p) c -> p i c", p=128))
                nc.sync.dma_start(out=wk_raw, in_=wk_d.rearrange("(i p) c -> p i c", p=128))
                nc.sync.dma_start(out=wv_raw, in_=wv_d.rearrange("(i p) c -> p i c", p=128))
                nc.sync.dma_start(out=wo_raw, in_=wo_d.rearrange("(i p) c -> p i c", p=128))
                nc.vector.tensor_copy(wq_sb, wq_raw)
                nc.vector.tensor_copy(wk_sb, wk_raw)
                nc.vector.tensor_copy(wv_sb, wv_raw)
                nc.vector.tensor_copy(woc, wo_raw)

            SC = 256  # s-chunk width
            # K side first: KpT + Vp (the first QK needs all of KpT)
            for sc in range(KL // SC):
                kraw = p1sb.tile([128, 8, SC], f32, tag="kraw", bufs=2)
                nc.sync.dma_start(
                    out=kraw,
                    in_=KT.rearrange("(i p) s -> p i s", p=128)[:, :, sc * SC:(sc + 1) * SC])
                if pr_dt == f32:
                    kch = kraw
                else:
                    kch = p1sb.tile([128, 8, SC], pr_dt, tag="kch", bufs=2)
                    nc.vector.tensor_copy(kch, kraw)
                for t in range(2):
                    ps2 = p1ps.tile([128, SC], f32, tag="pp")
                    for ki in range(8):
                        nc.tensor.matmul(
                            ps2,
                            lhsT=wk_sb[:, ki, t * 128:(t + 1) * 128],
                            rhs=kch[:, ki, :],
                            start=(ki == 0), stop=(ki == 7))
                    nc.scalar.activation(
                        out=kpt[t][:, sc * SC:(sc + 1) * SC], in_=ps2,
                        func=AF.Identity, bias=bk_sb[:, t:t + 1], scale=1.0)
                for st in range(SC // 128):
                    psv = p1ps.tile([128, CPC], f32, tag="ppv")
                    for ki in range(8):
                        nc.tensor.matmul(
                            psv,
                            lhsT=kch[:, ki, st * 128:(st + 1) * 128],
                            rhs=wv_sb[:, ki, :],
                            start=(ki == 0), stop=(ki == 7))
                    nc.vector.tensor_copy(vp[sc * (SC // 128) + st], psv)
            # Q side
            for sc in range(QL // SC):
                qraw = p1sb.tile([128, 8, SC], f32, tag="qraw", bufs=2)
                nc.sync.dma_start(
                    out=qraw,
                    in_=QT.rearrange("(i p) s -> p i s", p=128)[:, :, sc * SC:(sc + 1) * SC])
                if pr_dt == f32:
                    qch = qraw
                else:
                    qch = p1sb.tile([128, 8, SC], pr_dt, tag="qch", bufs=2)
                    nc.vector.tensor_copy(qch, qraw)
                for t in range(2):
                    ps = p1ps.tile([128, SC], f32, tag="pp")
                    for ki in range(8):
                        nc.tensor.matmul(
                            ps,
                            lhsT=wq_sb[:, ki, t * 128:(t + 1) * 128],
                            rhs=qch[:, ki, :],
                            start=(ki == 0), stop=(ki == 7))
                    nc.scalar.activation(
                        out=qpt[t][:, sc * SC:(sc + 1) * SC], in_=ps,
                        func=AF.Identity, bias=bq_sb[:, t:t + 1], scale=1.0)

        # ------------------------------------------------- phase 2: attention
        with tc.tile_pool(name="m2sb", bufs=1) as m2, \
             tc.tile_pool(name="m2ps", bufs=1, space="PSUM") as pp:
            for qc in range(4):
                mtiles = []
                for qt in range(4):
                    w_t = m2.tile([128, KL], f32, tag="wm", bufs=4)
                    qi = qc * 4 + qt
                    nc.sync.dma_start(out=w_t, in_=Wb[qi * 128:(qi + 1) * 128, :])
                    nc.scalar.activation(out=w_t, in_=w_t, func=AF.Sigmoid)
                    mtiles.append(w_t)
                for h in range(4):
                    hb = (h % 2) * 64
                    ht = h // 2
                    den_t = m2.tile([128, 4], f32, tag="den", bufs=2)
                    rec_t = m2.tile([128, 4], f32, tag="rec", bufs=2)
                    a_tiles = []
                    for qt in range(4):
                        qi = qc * 4 + qt
                        sm_t = m2.tile([128, KL], f32, tag="sm", bufs=2)
                        for half in range(2):
                            sp = pp.tile([128, 1024], f32, tag="sp", bufs=1)
                            for kc in range(2):
                                nc.tensor.matmul(
                                    sp[:, kc * 512:(kc + 1) * 512],
                                    lhsT=qpt[ht][hb:hb + 64, qi * 128:(qi + 1) * 128],
                                    rhs=kpt[ht][hb:hb + 64,
                                                half * 1024 + kc * 512:half * 1024 + (kc + 1) * 512],
                                    start=True, stop=True)
                            nc.vector.tensor_mul(
                                sm_t[:, half * 1024:(half + 1) * 1024],
                                sp, mtiles[qt][:, half * 1024:(half + 1) * 1024])
                        nc.scalar.activation(
                            out=sm_t, in_=sm_t, func=AF.Exp, scale=0.125,
                            accum_out=den_t[:, qt:qt + 1])
                        nc.vector.reciprocal(rec_t[:, qt:qt + 1], den_t[:, qt:qt + 1])
                        a_t = m2.tile([128, KL], f32, tag="a", bufs=8)
                        nc.vector.tensor_scalar_mul(a_t, sm_t, rec_t[:, qt:qt + 1])
                        nc.sync.dma_start(
                            out=attnw[h * QL + qi * 128:h * QL + (qi + 1) * 128, :],
                            in_=a_t)
                        a_tiles.append(a_t)
                    av = pp.tile([64, 512], f32, tag="av", bufs=1)
                    ci = 0
                    for kt in range(16):
                        at_t = m2.tile([128, 4, 128], av_dt, tag="at", bufs=3)
                        for pair in range(2):
                            atp = pp.tile([128, 1024], f32, tag="atp", bufs=2)
                            for j in range(2):
                                qt = pair * 2 + j
                                nc.tensor.transpose(
                                    out=atp[:, j * 512:j * 512 + 128],
                                    in_=a_tiles[qt][:, kt * 128:(kt + 1) * 128],
                                    identity=ident)
                            src = atp.rearrange("p (two c) -> p two c", two=2)[:, :, 0:128]
                            dst = at_t[:, pair * 2:pair * 2 + 2, :]
                            ci += 1
                            if ci % 9 < 4:
                                nc.vector.tensor_copy(dst, src)
                            else:
                                nc.scalar.copy(out=dst, in_=src)
                        nc.tensor.matmul(
                            av, lhsT=vp[kt][:, h * 64:(h + 1) * 64],
                            rhs=at_t.rearrange("p four c -> p (four c)"),
                            start=(kt == 0), stop=(kt == 15))
                    nc.scalar.copy(
                        out=aot[ht][hb:hb + 64, qc * 512:(qc + 1) * 512], in_=av)
                # out-projection for this qc's query tiles (aot cols final now)
                for st4 in range(4):
                    st = qc * 4 + st4
                    o_t = m2.tile([128, D], f32, tag="o", bufs=2)
                    for nb in range(2):
                        op = pp.tile([128, 512], f32, tag="op", bufs=1)
                        for ct in range(2):
                            nc.tensor.matmul(
                                op, lhsT=aot[ct][:, st * 128:(st + 1) * 128],
                                rhs=woc[:, ct, nb * 512:(nb + 1) * 512],
                                start=(ct == 0), stop=(ct == 1))
                        if nb == 0:
                            nc.vector.tensor_copy(o_t[:, 0:512], op)
                        else:
                            nc.scalar.copy(out=o_t[:, 512:1024], in_=op)
                    nc.sync.dma_start(out=partial[st * 128:(st + 1) * 128, :], in_=o_t)


    nc.finalize()
    return nc


def _get_nc(reps=1):
    key = (tuple(sorted(MM_CFG.items())), reps)
    if key not in _CACHE:
        _CACHE[key] = _build(key, reps=reps)
    return _CACHE[key]


def kernel(Q, K, W, Wq, bq, Wk, bk, Wv, bv, Wo, bo):
    from concourse.bass_utils import run_bass_kernel_spmd

    Q = np.asarray(Q, dtype=np.float32)
    K = np.asarray(K, dtype=np.float32)
    W = np.asarray(W, dtype=np.float32)
    Wq = np.asarray(Wq, dtype=np.float32)
    bq = np.asarray(bq, dtype=np.float32)
    Wk = np.asarray(Wk, dtype=np.float32)
    bk = np.asarray(bk, dtype=np.float32)
    Wv = np.asarray(Wv, dtype=np.float32)
    bv = np.asarray(bv, dtype=np.float32)
    Wo = np.asarray(Wo, dtype=np.float32)
    bo = np.asarray(bo, dtype=np.float32)

    nc = _get_nc()

    in_maps = []
    for c in range(NCORES):
        b = c // 4
        hg = c % 4
        cs = slice(hg * CPC, (hg + 1) * CPC)
        in_maps.append({
            "QT": np.ascontiguousarray(Q[b].T),
            "KT": np.ascontiguousarray(K[b].T),
            "Wb": np.ascontiguousarray(W[b]),
            "wq": np.ascontiguousarray(Wq[:, cs]),
            "wk": np.ascontiguousarray(Wk[:, cs]),
            "wv": np.ascontiguousarray(Wv[:, cs]),
            "wo": np.ascontiguousarray(Wo[cs, :]),
            "bq2": np.ascontiguousarray(bq[cs].reshape(2, 128).T),
            "bk2": np.ascontiguousarray(bk[cs].reshape(2, 128).T),
        })

    global LAST_IN_MAPS
    LAST_IN_MAPS = in_maps
    res = run_bass_kernel_spmd(nc, in_maps, list(range(NCORES))).results

    bo_full = bo + bv @ Wo
    output = np.empty((B, QL, D), np.float32)
    attn = np.empty((B, NH, QL, KL), np.float32)
    for b in range(B):
        acc = None
        for hg in range(4):
            c = b * 4 + hg
            p = res[c]["partial"]
            acc = p.copy() if acc is None else acc + p
            attn[b, hg * HPC:(hg + 1) * HPC] = res[c]["attnw"].reshape(HPC, QL, KL)
        output[b] = acc + bo_full[None, :]
    return output, attn
